# revision 15
# baseline (speedup 1.0000x reference)
"""Trainium2 Bass kernel for nn_ActQuantizerGS (grouped int4 activation
quantizer with per-group Hadamard rotation).

Reference semantics (group size 128, QMAX=7):
    x2d = x.reshape(-1, 128)
    h = x2d @ H                          (H = unnormalized Sylvester Hadamard)
    amax = max|h| per group
    scale = amax/7 (1.0 if amax==0)
    qi = clip(round(h/scale), -8, 7)     (|h/scale| <= 7+eps so the clip
                                          never binds beyond +-7)
    q = packed nibble pairs (low = even index, high = odd index)
    scales_out = scale / sqrt(128)
    zero_point = zeros(uint8)

Distribution: data-parallel across 8 NeuronCores over the flattened row dim
(8192 rows of 8192 features -> 1024 rows per core).  No communication.

Per-core dataflow, per 128-row tile, per batch of 8 groups:
    PE : 8x transpose(x_blk) -> PSUM                   (fp32 is_transpose)
    ACT: batched PSUM->SBUF copy of the transposes
    PE : 8x h = matmul(lhsT=xT_blk, rhs=H_perm) -> PSUM (H columns permuted
         so even outputs land in cols 0..63, odd in 64..127)
    DVE: grouped abs-max reduce of h                   (one op per batch)
    GPS: spre = amax * (1/7)
    DVE: inv = reciprocal(spre)
    quant to int8 codes qi = rne(h * inv), either
      DVE: one broadcast tensor_tensor(mult) per batch (step-0 AP on inv), or
      ACT: 8 per-block activations (Copy, scale=inv) -- batches split across
           the two engines for load balance.  Both convert with RNE.
    per row tile:
      GPS: scales = spre * (1/sqrt(128))
      DVE: pack int32-view: q = (A32 & 0x0F0F0F0F) | ((B32<<4) & 0xF0F0F0F0)
      DMA: q / scales / zero_point stores.
"""
import os
import sys
import numpy as np

try:
    import concourse.bass as bass
except ImportError:  # fresh interpreter without the default PYTHONPATH
    for p in ("/opt/trn_rl_repo", "/root/.axon_site/_ro/trn_rl_repo"):
        if os.path.isdir(p) and p not in sys.path:
            sys.path.insert(0, p)
    import concourse.bass as bass

import concourse.tile as tile
from concourse import bacc, mybir
from concourse.bass_utils import run_bass_kernel_spmd

dt = mybir.dt

# Problem geometry (hardcoded per the task contract).
B, S, F = 2, 4096, 8192
GS = 128                       # hadamard group size
N_CORES = 8
ROWS = B * S                   # 8192 flattened rows
ROWS_PER_CORE = ROWS // N_CORES     # 1024
RT_PER_CORE = ROWS_PER_CORE // 128  # 8 row tiles of 128 rows
NG = F // GS                   # 64 groups per row
NB = 8                         # groups per batch (= one 2-bank psum tile)
NBATCH = NG // NB              # 8 batches per row tile
QB = F // 2                    # 4096 packed bytes per row

SQRT_GS_F32 = np.float32(np.sqrt(np.float64(GS)))
INV7 = float(np.float32(1.0) / np.float32(7.0))
INV_SQRT = float(np.float32(1.0) / SQRT_GS_F32)
MASK_LO = float(0x0F0F0F0F)
MASK_HI = float(np.int32(np.uint32(0xF0F0F0F0).astype(np.int32)))

# Batches whose quant step runs as 8 per-block ACT ops instead of one DVE
# broadcast multiply (load balancing: ACT op ~0.55us, DVE batch ~1.4us).
ACT_QUANT_BATCHES = (1, 4, 6)  # non-adjacent, away from row-tile boundaries


def _hadamard(n):
    H = np.array([[1.0]], dtype=np.float32)
    while H.shape[0] < n:
        H = np.block([[H, H], [H, -H]]).astype(np.float32)
    return H


def _build_consts():
    H = _hadamard(GS)
    perm = np.concatenate([np.arange(0, GS, 2), np.arange(1, GS, 2)])
    hp = H[:, perm].astype(np.float32)            # [128, 128]
    ident = np.eye(GS, dtype=np.float32)          # [128, 128]
    return np.concatenate([hp, ident], axis=1)    # [128, 256]


def _build_kernel():
    nc = bacc.Bacc("TRN2", target_bir_lowering=False, debug=False)

    x_d = nc.dram_tensor("x", [ROWS_PER_CORE, F], dt.float32, kind="ExternalInput")
    c_d = nc.dram_tensor("consts", [GS, 256], dt.float32, kind="ExternalInput")
    q_d = nc.dram_tensor("q", [ROWS_PER_CORE, QB], dt.uint8, kind="ExternalOutput")
    s_d = nc.dram_tensor("scales", [ROWS_PER_CORE, NG], dt.float32,
                         kind="ExternalOutput")
    z_d = nc.dram_tensor("zp", [ROWS_PER_CORE, NG], dt.uint8, kind="ExternalOutput")

    with tile.TileContext(nc) as tc:
        with (
            tc.tile_pool(name="cst", bufs=1) as cst,
            tc.tile_pool(name="xin", bufs=2) as xin,
            tc.tile_pool(name="tps", bufs=2, space="PSUM") as tpsp,
            tc.tile_pool(name="hps", bufs=3, space="PSUM") as hpsp,
            tc.tile_pool(name="xts", bufs=3) as xts,
            tc.tile_pool(name="sml", bufs=4) as sml,
            tc.tile_pool(name="qip", bufs=2) as qip,
            tc.tile_pool(name="pkp", bufs=2) as pkp,
            tc.tile_pool(name="outp", bufs=2) as outp,
        ):
            consts = cst.tile([GS, 256], dt.float32)
            nc.gpsimd.dma_start(consts[:], c_d.ap())
            hp_t = consts[:, 0:128]
            id_t = consts[:, 128:256]

            zp_t = cst.tile([128, NG], dt.uint8)
            nc.vector.memset(zp_t[:], 0)

            # x row tiles are prefetched one row-tile ahead, in halves so the
            # first batches of a row tile can start before the whole 4MB lands.
            FQ = F // 4

            def load_x(rt, split_head=False):
                t = xin.tile([128, F], dt.float32, tag="xrt")
                r0 = rt * 128
                if split_head:
                    # first batch (groups 0-7 = cols 0:1024) lands early
                    nc.sync.dma_start(t[:, 0:1024], x_d.ap()[r0:r0 + 128, 0:1024])
                    nc.sync.dma_start(t[:, 1024:FQ], x_d.ap()[r0:r0 + 128, 1024:FQ])
                else:
                    nc.sync.dma_start(t[:, 0:FQ], x_d.ap()[r0:r0 + 128, 0:FQ])
                for qtr in range(1, 4):
                    nc.sync.dma_start(
                        t[:, qtr * FQ:(qtr + 1) * FQ],
                        x_d.ap()[r0:r0 + 128, qtr * FQ:(qtr + 1) * FQ])
                return t

            x_tiles = {0: load_x(0, split_head=True)}

            for rt in range(RT_PER_CORE):
                x_rt = x_tiles.pop(rt)
                if rt + 1 < RT_PER_CORE:
                    x_tiles[rt + 1] = load_x(rt + 1)

                qi_rt = qip.tile([128, F], dt.int8, tag="qirt")
                spre_rt = sml.tile([128, NG], dt.float32, tag="sprert")
                sc_rt = outp.tile([128, NG], dt.float32, tag="scrt")

                # software pipeline: matmuls of batch b interleave with the
                # transposes of batch b+1 (MM first so the PE FIFO is never
                # blocked by a transpose waiting on staging space), keeping
                # the PE dense and the HAM clock governor warm.  Transpose
                # staging is two 1-bank psum tiles per batch; h batches are
                # 2-bank psum tiles, triple buffered.
                tps_t = {}
                xt_t = {}
                hps_t = {}

                def emit_transpose_half(b, hf):
                    tps = tpsp.tile([128, 512], dt.float32, tag="tps",
                                    name=f"tps_{rt}_{b}_{hf}")
                    tps_t[(b, hf)] = tps
                    for j in range(4):
                        g = b * NB + hf * 4 + j
                        nc.tensor.transpose(
                            tps[:, j * 128:(j + 1) * 128],
                            x_rt[:, g * 128:(g + 1) * 128],
                            id_t,
                        )

                def emit_copy_half(b, hf):
                    if (b, "t") not in xt_t:
                        xt_t[(b, "t")] = xts.tile([128, NB * 128], dt.float32,
                                                  tag="xts",
                                                  name=f"xt_{rt}_{b}")
                    xt_sb = xt_t[(b, "t")]
                    nc.scalar.copy(xt_sb[:, hf * 512:(hf + 1) * 512],
                                   tps_t.pop((b, hf))[:])

                emit_transpose_half(0, 0)
                emit_copy_half(0, 0)
                emit_transpose_half(0, 1)
                emit_copy_half(0, 1)

                for bi in range(NBATCH):
                    hps = hpsp.tile([128, NB * 128], dt.float32, tag="hps",
                                    name=f"hps_{rt}_{bi}")
                    hps_t[bi] = hps
                    xt_sb = xt_t.pop((bi, "t"))
                    has_next = bi + 1 < NBATCH
                    for j in range(NB):
                        nc.tensor.matmul(
                            hps[:, j * 128:(j + 1) * 128],
                            xt_sb[:, j * 128:(j + 1) * 128],
                            hp_t,
                        )
                        if has_next:
                            if j == 3:
                                emit_transpose_half(bi + 1, 0)
                                emit_copy_half(bi + 1, 0)
                            elif j == 7:
                                emit_transpose_half(bi + 1, 1)
                                emit_copy_half(bi + 1, 1)

                    amax8 = sml.tile([128, NB], dt.float32, tag="amax")
                    nc.vector.tensor_reduce(
                        amax8[:],
                        hps[:].rearrange("p (g e) -> p g e", e=128),
                        axis=mybir.AxisListType.X,
                        op=mybir.AluOpType.max,
                        apply_absolute_value=True,
                    )
                    # spre = amax * (1/7) (gpsimd; within 1 ulp of amax/7)
                    spre8 = spre_rt[:, bi * NB:(bi + 1) * NB]
                    nc.gpsimd.tensor_scalar(
                        spre8, amax8[:], INV7, 1e-37,
                        op0=mybir.AluOpType.mult,
                        op1=mybir.AluOpType.max,
                    )
                    inv8 = sml.tile([128, NB], dt.float32, tag="inv")
                    nc.vector.reciprocal(inv8[:], spre8)

                    qi_b = qi_rt[:, bi * NB * 128:(bi + 1) * NB * 128]
                    if bi in ACT_QUANT_BATCHES:
                        for j in range(NB):
                            nc.scalar.activation(
                                qi_b[:, j * 128:(j + 1) * 128],
                                hps[:, j * 128:(j + 1) * 128],
                                mybir.ActivationFunctionType.Copy,
                                bias=0.0,
                                scale=inv8[:, j:j + 1],
                            )
                    else:
                        inv_bc = inv8[:].rearrange(
                            "p (g o) -> p g o", o=1).broadcast_to([128, NB, 128])
                        nc.vector.tensor_tensor(
                            out=qi_b.rearrange("p (g e) -> p g e", e=128),
                            in0=hps[:].rearrange("p (g e) -> p g e", e=128),
                            in1=inv_bc,
                            op=mybir.AluOpType.mult,
                        )
                    hps_t.pop(bi)

                # scales = spre * (1/sqrt(128))
                nc.gpsimd.tensor_scalar(
                    sc_rt[:], spre_rt[:], INV_SQRT, None,
                    op0=mybir.AluOpType.mult,
                )

                # pack: q = (A & 0x0F0F0F0F) | ((B << 4) & 0xF0F0F0F0),
                # done in half-row-tile chunks (quarters on the last row tile
                # so the kernel tail is short)
                q_rt = outp.tile([128, QB], dt.uint8, tag="qrt")
                nchunk = 2 if rt + 1 < RT_PER_CORE else 4
                NGH = NG // nchunk
                for hf in range(nchunk):
                    qw = QB // nchunk
                    qi_h = qi_rt[:, hf * NGH * 128:(hf + 1) * NGH * 128]
                    q32 = qi_h.bitcast(dt.int32)
                    a_w = q32.rearrange("p (g w) -> p g w", w=32)[:, :, 0:16]
                    b_w = q32.rearrange("p (g w) -> p g w", w=32)[:, :, 16:32]
                    pa = pkp.tile([128, qw // 4], dt.int32, tag="pa",
                                  name=f"pa_{rt}_{hf}")
                    nc.vector.tensor_scalar(
                        pa[:], a_w, MASK_LO, None,
                        op0=mybir.AluOpType.bitwise_and,
                    )
                    pb = pkp.tile([128, qw // 4], dt.int32, tag="pb",
                                  name=f"pb_{rt}_{hf}")
                    nc.vector.tensor_scalar(
                        pb[:], b_w, 4.0, MASK_HI,
                        op0=mybir.AluOpType.logical_shift_left,
                        op1=mybir.AluOpType.bitwise_and,
                    )
                    nc.vector.tensor_tensor(
                        out=q_rt[:, hf * qw:(hf + 1) * qw].bitcast(dt.int32),
                        in0=pa[:],
                        in1=pb[:],
                        op=mybir.AluOpType.bitwise_or,
                    )
                    # store each packed chunk as soon as it is ready (sync
                    # queue; the x prefetch for rt+1 was issued earlier so
                    # only the rt+2 load sits behind these, with a full
                    # row-tile of slack)
                    r0, r1 = rt * 128, (rt + 1) * 128
                    nc.sync.dma_start(
                        q_d.ap()[r0:r1, hf * qw:(hf + 1) * qw],
                        q_rt[:, hf * qw:(hf + 1) * qw])

                nc.sync.dma_start(s_d.ap()[r0:r1, :], sc_rt[:])
                nc.sync.dma_start(z_d.ap()[r0:r1, :], zp_t[:])

    nc.compile()
    return nc


_CACHE = {}


def _get_kernel():
    if "nc" not in _CACHE:
        _CACHE["nc"] = _build_kernel()
        _CACHE["consts"] = _build_consts()
    return _CACHE["nc"], _CACHE["consts"]


def kernel(x):
    x = np.asarray(x, dtype=np.float32)
    assert x.shape == (B, S, F), f"unexpected shape {x.shape}"
    nc, consts = _get_kernel()

    x2 = x.reshape(ROWS, F)
    in_maps = [
        {
            "x": x2[c * ROWS_PER_CORE:(c + 1) * ROWS_PER_CORE],
            "consts": consts,
        }
        for c in range(N_CORES)
    ]
    trace = bool(os.environ.get("KERNEL_TRACE"))
    res = run_bass_kernel_spmd(nc, in_maps, list(range(N_CORES)), trace=trace)
    if trace:
        print(f"HW exec time: {res.exec_time_ns} ns")
        _CACHE["exec_time_ns"] = res.exec_time_ns

    q = np.concatenate([res.results[c]["q"] for c in range(N_CORES)], axis=0)
    sc = np.concatenate([res.results[c]["scales"] for c in range(N_CORES)], axis=0)
    zp = np.concatenate([res.results[c]["zp"] for c in range(N_CORES)], axis=0)

    scales = sc.reshape(B, S, NG).astype(np.float32)
    zero_point = zp.reshape(B, S, NG).astype(np.uint8)
    qq = q.reshape(B, S, QB).astype(np.uint8)
    return scales, zero_point, qq


# revision 16
# speedup vs baseline: 1.0036x; 1.0036x over previous
"""Trainium2 Bass kernel for nn_ActQuantizerGS (grouped int4 activation
quantizer with per-group Hadamard rotation).

Reference semantics (group size 128, QMAX=7):
    x2d = x.reshape(-1, 128)
    h = x2d @ H                          (H = unnormalized Sylvester Hadamard)
    amax = max|h| per group
    scale = amax/7 (1.0 if amax==0)
    qi = clip(round(h/scale), -8, 7)     (|h/scale| <= 7+eps so the clip
                                          never binds beyond +-7)
    q = packed nibble pairs (low = even index, high = odd index)
    scales_out = scale / sqrt(128)
    zero_point = zeros(uint8)

Distribution: data-parallel across 8 NeuronCores over the flattened row dim
(8192 rows of 8192 features -> 1024 rows per core).  No communication.

Per-core dataflow, per 128-row tile, per batch of 8 groups:
    PE : 8x transpose(x_blk) -> PSUM                   (fp32 is_transpose)
    ACT: batched PSUM->SBUF copy of the transposes
    PE : 8x h = matmul(lhsT=xT_blk, rhs=H_perm) -> PSUM (H columns permuted
         so even outputs land in cols 0..63, odd in 64..127)
    DVE: grouped abs-max reduce of h                   (one op per batch)
    GPS: spre = amax * (1/7)
    DVE: inv = reciprocal(spre)
    quant to int8 codes qi = rne(h * inv), either
      DVE: one broadcast tensor_tensor(mult) per batch (step-0 AP on inv), or
      ACT: 8 per-block activations (Copy, scale=inv) -- batches split across
           the two engines for load balance.  Both convert with RNE.
    per row tile:
      GPS: scales = spre * (1/sqrt(128))
      DVE: pack int32-view: q = (A32 & 0x0F0F0F0F) | ((B32<<4) & 0xF0F0F0F0)
      DMA: q / scales / zero_point stores.
"""
import os
import sys
import numpy as np

try:
    import concourse.bass as bass
except ImportError:  # fresh interpreter without the default PYTHONPATH
    for p in ("/opt/trn_rl_repo", "/root/.axon_site/_ro/trn_rl_repo"):
        if os.path.isdir(p) and p not in sys.path:
            sys.path.insert(0, p)
    import concourse.bass as bass

import concourse.tile as tile
from concourse import bacc, mybir
from concourse.bass_utils import run_bass_kernel_spmd

dt = mybir.dt

# Problem geometry (hardcoded per the task contract).
B, S, F = 2, 4096, 8192
GS = 128                       # hadamard group size
N_CORES = 8
ROWS = B * S                   # 8192 flattened rows
ROWS_PER_CORE = ROWS // N_CORES     # 1024
RT_PER_CORE = ROWS_PER_CORE // 128  # 8 row tiles of 128 rows
NG = F // GS                   # 64 groups per row
NB = 8                         # groups per batch (= one 2-bank psum tile)
NBATCH = NG // NB              # 8 batches per row tile
QB = F // 2                    # 4096 packed bytes per row

SQRT_GS_F32 = np.float32(np.sqrt(np.float64(GS)))
INV7 = float(np.float32(1.0) / np.float32(7.0))
INV_SQRT = float(np.float32(1.0) / SQRT_GS_F32)
MASK_LO = float(0x0F0F0F0F)
MASK_HI = float(np.int32(np.uint32(0xF0F0F0F0).astype(np.int32)))

# Batches whose quant step runs as 8 per-block ACT ops instead of one DVE
# broadcast multiply (load balancing: ACT op ~0.55us, DVE batch ~1.4us).
ACT_QUANT_BATCHES = (1, 4, 6)  # non-adjacent, away from row-tile boundaries


def _hadamard(n):
    H = np.array([[1.0]], dtype=np.float32)
    while H.shape[0] < n:
        H = np.block([[H, H], [H, -H]]).astype(np.float32)
    return H


def _build_consts():
    H = _hadamard(GS)
    perm = np.concatenate([np.arange(0, GS, 2), np.arange(1, GS, 2)])
    hp = H[:, perm].astype(np.float32)            # [128, 128]
    ident = np.eye(GS, dtype=np.float32)          # [128, 128]
    return np.concatenate([hp, ident], axis=1)    # [128, 256]


def _build_kernel():
    nc = bacc.Bacc("TRN2", target_bir_lowering=False, debug=False)

    x_d = nc.dram_tensor("x", [ROWS_PER_CORE, F], dt.float32, kind="ExternalInput")
    c_d = nc.dram_tensor("consts", [GS, 256], dt.float32, kind="ExternalInput")
    q_d = nc.dram_tensor("q", [ROWS_PER_CORE, QB], dt.uint8, kind="ExternalOutput")
    s_d = nc.dram_tensor("scales", [ROWS_PER_CORE, NG], dt.float32,
                         kind="ExternalOutput")
    z_d = nc.dram_tensor("zp", [ROWS_PER_CORE, NG], dt.uint8, kind="ExternalOutput")

    with tile.TileContext(nc) as tc:
        with (
            tc.tile_pool(name="cst", bufs=1) as cst,
            tc.tile_pool(name="xin", bufs=2) as xin,
            tc.tile_pool(name="tps", bufs=2, space="PSUM") as tpsp,
            tc.tile_pool(name="hps", bufs=3, space="PSUM") as hpsp,
            tc.tile_pool(name="xts", bufs=3) as xts,
            tc.tile_pool(name="sml", bufs=4) as sml,
            tc.tile_pool(name="qip", bufs=2) as qip,
            tc.tile_pool(name="pkp", bufs=2) as pkp,
            tc.tile_pool(name="outp", bufs=2) as outp,
        ):
            consts = cst.tile([GS, 256], dt.float32)
            nc.gpsimd.dma_start(consts[:], c_d.ap())
            hp_t = consts[:, 0:128]
            id_t = consts[:, 128:256]

            zp_t = cst.tile([128, NG], dt.uint8)
            nc.vector.memset(zp_t[:], 0)

            # x row tiles are prefetched one row-tile ahead, in halves so the
            # first batches of a row tile can start before the whole 4MB lands.
            FQ = F // 4

            def load_x(rt, split_head=False):
                t = xin.tile([128, F], dt.float32, tag="xrt")
                r0 = rt * 128
                if split_head:
                    # first batch (groups 0-7 = cols 0:1024) lands early
                    nc.sync.dma_start(t[:, 0:1024], x_d.ap()[r0:r0 + 128, 0:1024])
                    nc.sync.dma_start(t[:, 1024:FQ], x_d.ap()[r0:r0 + 128, 1024:FQ])
                else:
                    nc.sync.dma_start(t[:, 0:FQ], x_d.ap()[r0:r0 + 128, 0:FQ])
                for qtr in range(1, 4):
                    nc.sync.dma_start(
                        t[:, qtr * FQ:(qtr + 1) * FQ],
                        x_d.ap()[r0:r0 + 128, qtr * FQ:(qtr + 1) * FQ])
                return t

            x_tiles = {0: load_x(0, split_head=True)}

            NTOT = RT_PER_CORE * NBATCH  # 64 batches, flat pipeline
            tps_t = {}
            xt_t = {}
            rt_state = {}

            def get_x(k):
                return x_tiles[k // NBATCH]

            def emit_transpose_one(k, j):
                if k not in tps_t:
                    hf = j // 4
                    # two 1-bank staging tiles per batch
                    tps_t[k] = [None, None]
                if tps_t[k][j // 4] is None:
                    tps_t[k][j // 4] = tpsp.tile(
                        [128, 512], dt.float32, tag="tps",
                        name=f"tps_{k}_{j // 4}")
                g = (k % NBATCH) * NB + j
                nc.tensor.transpose(
                    tps_t[k][j // 4][:, (j % 4) * 128:(j % 4 + 1) * 128],
                    get_x(k)[:, g * 128:(g + 1) * 128],
                    id_t,
                )

            def emit_copy_half(k, hf):
                if k not in xt_t:
                    xt_t[k] = xts.tile([128, NB * 128], dt.float32,
                                       tag="xts", name=f"xt_{k}")
                nc.scalar.copy(xt_t[k][:, hf * 512:(hf + 1) * 512],
                               tps_t[k][hf][:])
                tps_t[k][hf] = False

            def start_rt(rt):
                st = {}
                st["qi"] = qip.tile([128, F], dt.int8, tag="qirt",
                                    name=f"qi_{rt}")
                st["spre"] = sml.tile([128, NG], dt.float32, tag="sprert",
                                      name=f"spre_{rt}")
                st["sc"] = outp.tile([128, NG], dt.float32, tag="scrt",
                                     name=f"sc_{rt}")
                rt_state[rt] = st
                return st

            def finish_rt(rt):
                st = rt_state.pop(rt)
                qi_rt = st["qi"]
                sc_rt = st["sc"]
                spre_rt = st["spre"]
                # scales = spre * (1/sqrt(128))
                nc.gpsimd.tensor_scalar(
                    sc_rt[:], spre_rt[:], INV_SQRT, None,
                    op0=mybir.AluOpType.mult,
                )
                # pack: q = (A & 0x0F0F0F0F) | ((B << 4) & 0xF0F0F0F0)
                q_rt = outp.tile([128, QB], dt.uint8, tag="qrt",
                                 name=f"q_{rt}")
                nchunk = 2 if rt + 1 < RT_PER_CORE else 4
                NGH = NG // nchunk
                for hf in range(nchunk):
                    qw = QB // nchunk
                    qi_h = qi_rt[:, hf * NGH * 128:(hf + 1) * NGH * 128]
                    q32 = qi_h.bitcast(dt.int32)
                    a_w = q32.rearrange("p (g w) -> p g w", w=32)[:, :, 0:16]
                    b_w = q32.rearrange("p (g w) -> p g w", w=32)[:, :, 16:32]
                    pa = pkp.tile([128, qw // 4], dt.int32, tag="pa",
                                  name=f"pa_{rt}_{hf}")
                    nc.vector.tensor_scalar(
                        pa[:], a_w, MASK_LO, None,
                        op0=mybir.AluOpType.bitwise_and,
                    )
                    pb = pkp.tile([128, qw // 4], dt.int32, tag="pb",
                                  name=f"pb_{rt}_{hf}")
                    nc.vector.tensor_scalar(
                        pb[:], b_w, 4.0, MASK_HI,
                        op0=mybir.AluOpType.logical_shift_left,
                        op1=mybir.AluOpType.bitwise_and,
                    )
                    nc.vector.tensor_tensor(
                        out=q_rt[:, hf * qw:(hf + 1) * qw].bitcast(dt.int32),
                        in0=pa[:],
                        in1=pb[:],
                        op=mybir.AluOpType.bitwise_or,
                    )
                    r0, r1 = rt * 128, (rt + 1) * 128
                    nc.sync.dma_start(
                        q_d.ap()[r0:r1, hf * qw:(hf + 1) * qw],
                        q_rt[:, hf * qw:(hf + 1) * qw])
                nc.sync.dma_start(s_d.ap()[r0:r1, :], sc_rt[:])
                nc.sync.dma_start(z_d.ap()[r0:r1, :], zp_t[:])

            # pipeline prologue: batch 0 transposes + copies
            for j in range(4):
                emit_transpose_one(0, j)
            emit_copy_half(0, 0)
            for j in range(4, 8):
                emit_transpose_one(0, j)
            emit_copy_half(0, 1)

            for k in range(NTOT):
                rt, bi = divmod(k, NBATCH)
                if bi == 0:
                    st = start_rt(rt)
                    if rt + 1 < RT_PER_CORE:
                        x_tiles[rt + 1] = load_x(rt + 1)
                qi_rt = rt_state[rt]["qi"]
                spre_rt = rt_state[rt]["spre"]

                hps = hpsp.tile([128, NB * 128], dt.float32, tag="hps",
                                name=f"hps_{k}")
                xt_sb = xt_t.pop(k)
                has_next = k + 1 < NTOT
                # interleave: 4 matmuls of batch k, then 4 transposes of
                # batch k+1 + a half-copy, then the other 4+4
                for j in range(NB):
                    nc.tensor.matmul(
                        hps[:, j * 128:(j + 1) * 128],
                        xt_sb[:, j * 128:(j + 1) * 128],
                        hp_t,
                    )
                    if has_next:
                        if j == 3:
                            for jj in range(4):
                                emit_transpose_one(k + 1, jj)
                            emit_copy_half(k + 1, 0)
                        elif j == 7:
                            for jj in range(4, 8):
                                emit_transpose_one(k + 1, jj)
                            emit_copy_half(k + 1, 1)

                amax8 = sml.tile([128, NB], dt.float32, tag="amax",
                                 name=f"amax_{k}")
                nc.vector.tensor_reduce(
                    amax8[:],
                    hps[:].rearrange("p (g e) -> p g e", e=128),
                    axis=mybir.AxisListType.X,
                    op=mybir.AluOpType.max,
                    apply_absolute_value=True,
                )
                spre8 = spre_rt[:, bi * NB:(bi + 1) * NB]
                nc.gpsimd.tensor_scalar(
                    spre8, amax8[:], INV7, 1e-37,
                    op0=mybir.AluOpType.mult,
                    op1=mybir.AluOpType.max,
                )
                inv8 = sml.tile([128, NB], dt.float32, tag="inv",
                                name=f"inv_{k}")
                nc.vector.reciprocal(inv8[:], spre8)

                qi_b = qi_rt[:, bi * NB * 128:(bi + 1) * NB * 128]
                if bi in ACT_QUANT_BATCHES:
                    for j in range(NB):
                        nc.scalar.activation(
                            qi_b[:, j * 128:(j + 1) * 128],
                            hps[:, j * 128:(j + 1) * 128],
                            mybir.ActivationFunctionType.Copy,
                            bias=0.0,
                            scale=inv8[:, j:j + 1],
                        )
                else:
                    inv_bc = inv8[:].rearrange(
                        "p (g o) -> p g o", o=1).broadcast_to([128, NB, 128])
                    nc.vector.tensor_tensor(
                        out=qi_b.rearrange("p (g e) -> p g e", e=128),
                        in0=hps[:].rearrange("p (g e) -> p g e", e=128),
                        in1=inv_bc,
                        op=mybir.AluOpType.mult,
                    )

                if bi == NBATCH - 1:
                    x_tiles.pop(rt, None)
                    finish_rt(rt)

    nc.compile()
    return nc


_CACHE = {}


def _get_kernel():
    if "nc" not in _CACHE:
        _CACHE["nc"] = _build_kernel()
        _CACHE["consts"] = _build_consts()
    return _CACHE["nc"], _CACHE["consts"]


def kernel(x):
    x = np.asarray(x, dtype=np.float32)
    assert x.shape == (B, S, F), f"unexpected shape {x.shape}"
    nc, consts = _get_kernel()

    x2 = x.reshape(ROWS, F)
    in_maps = [
        {
            "x": x2[c * ROWS_PER_CORE:(c + 1) * ROWS_PER_CORE],
            "consts": consts,
        }
        for c in range(N_CORES)
    ]
    trace = bool(os.environ.get("KERNEL_TRACE"))
    res = run_bass_kernel_spmd(nc, in_maps, list(range(N_CORES)), trace=trace)
    if trace:
        print(f"HW exec time: {res.exec_time_ns} ns")
        _CACHE["exec_time_ns"] = res.exec_time_ns

    q = np.concatenate([res.results[c]["q"] for c in range(N_CORES)], axis=0)
    sc = np.concatenate([res.results[c]["scales"] for c in range(N_CORES)], axis=0)
    zp = np.concatenate([res.results[c]["zp"] for c in range(N_CORES)], axis=0)

    scales = sc.reshape(B, S, NG).astype(np.float32)
    zero_point = zp.reshape(B, S, NG).astype(np.uint8)
    qq = q.reshape(B, S, QB).astype(np.uint8)
    return scales, zero_point, qq
